# revision 3
# baseline (speedup 1.0000x reference)
"""Trainium2 Bass kernel for the gnn_message_passing problem.

Math reduction: the reference builds a [8192,8192] zero-diagonal adjacency
W_full from per-node Linear(8191,1) weights, forms state = [x | zeros] and
returns (state @ W_full.T + bias)[:, 7168:][:, ::-1].

Because state is zero outside its first 1024 columns, and only output nodes
7168..8191 are read, the whole computation collapses to

    out[b, k] = sum_c x[b, c] * weights[8191-k, c] + bias[8191-k]

i.e. a [32,1024] x [1024,1024]^T matmul + bias (for rows n >= 7168 and
cols c < 1024 we always have c < n, so W_full[n, c] == weights[n, c]).

Distribution: shard the 1024 output features row-wise across 8 cores
(128 each, tensor parallel); every core holds the replicated x. No
collectives — the host concatenates the 8 output slices.

Per-core Bass kernel: out_slice[k', b] = sum_c W_slice[k', c] * xT[c, b]
as 8 PSUM-accumulated bf16 matmuls over the contraction dim (1024).
Weights/x are cast to bf16 on host (measured rel err ~2.4e-3 vs the f32
reference, well under the 2e-2 gate) to halve the dominant DMA traffic.
The weight load is split into two DMAs on the SP HWDGE ring while x/bias
ride the ACT ring, so the first half's matmuls overlap the second half's
transfer. Bias is added on the scalar engine during the PSUM->SBUF copy.
"""

import numpy as np
import ml_dtypes

import concourse.bacc as bacc
import concourse.bass as bass
import concourse.mybir as mybir
from concourse.bass_utils import run_bass_kernel_spmd
from concourse.tile import TileContext

NODES = 8192
IN_F = 1024
OUT_F = 1024
B = 32
N_CORES = 8
KPC = OUT_F // N_CORES   # output features per core: 128
NCHUNK = IN_F // 128     # contraction chunks: 8
NSPLIT = 2               # weight DMA chunks
CPS = NCHUNK // NSPLIT   # contraction chunks per weight DMA: 4

F32 = mybir.dt.float32
BF16 = mybir.dt.bfloat16

_NC = None
LAST_RESULT = None  # BassKernelResults of the most recent run (for profiling)


def _build_nc():
    nc = bacc.Bacc(None, target_bir_lowering=False)

    # Per-core inputs, pre-packed on host so partition dim is contiguous:
    #   wt[p, n*KPC + k'] = W_eff[core*KPC + k', n*128 + p]   (bf16)
    #   xt[p, n*B   + b ] = x[b, n*128 + p]                   (bf16)
    wt = nc.dram_tensor("wt", [128, NCHUNK * KPC], BF16, kind="ExternalInput")
    xt = nc.dram_tensor("xt", [128, NCHUNK * B], BF16, kind="ExternalInput")
    bi = nc.dram_tensor("bi", [KPC, 1], F32, kind="ExternalInput")
    out = nc.dram_tensor("out", [KPC, B], F32, kind="ExternalOutput")

    with TileContext(nc) as tc:
        with (
            tc.tile_pool(name="sbuf", bufs=1) as pool,
            tc.tile_pool(name="psum", bufs=1, space=bass.MemorySpace.PSUM) as psum_pool,
        ):
            wt_ts = [
                pool.tile([128, CPS * KPC], BF16, name=f"wt{s}", tag=f"wt{s}")
                for s in range(NSPLIT)
            ]
            xt_t = pool.tile([128, NCHUNK * B], BF16)
            b_t = pool.tile([KPC, 1], F32)
            o_t = pool.tile([KPC, B], F32)
            ps = psum_pool.tile([KPC, B], F32)

            # Weight halves stream on the SP ring; x/bias on the ACT ring.
            nc.sync.dma_start(wt_ts[0][:], wt[:, : CPS * KPC])
            nc.scalar.dma_start(xt_t[:], xt[:])
            nc.scalar.dma_start(b_t[:], bi[:])
            for s in range(1, NSPLIT):
                nc.sync.dma_start(
                    wt_ts[s][:], wt[:, s * CPS * KPC : (s + 1) * CPS * KPC]
                )

            for n in range(NCHUNK):
                s, j = divmod(n, CPS)
                nc.tensor.matmul(
                    ps[:],
                    wt_ts[s][:, j * KPC : (j + 1) * KPC],  # lhsT [c=128, k'=128]
                    xt_t[:, n * B : (n + 1) * B],          # rhs  [c=128, b=32]
                    start=(n == 0),
                    stop=(n == NCHUNK - 1),
                )

            nc.scalar.activation(
                o_t[:], ps[:], mybir.ActivationFunctionType.Identity, bias=b_t[:]
            )
            nc.sync.dma_start(out[:], o_t[:])

    nc.finalize()
    return nc


def kernel(x: np.ndarray, weights: np.ndarray, bias: np.ndarray) -> np.ndarray:
    global _NC, LAST_RESULT
    if _NC is None:
        _NC = _build_nc()

    x = np.asarray(x, dtype=np.float32)
    weights = np.asarray(weights, dtype=np.float32)
    bias = np.asarray(bias, dtype=np.float32)

    # Effective dense weight block and bias (see module docstring).
    w_eff = weights[NODES - OUT_F :, :IN_F][::-1]  # [1024 (k), 1024 (c)]
    b_eff = bias[NODES - OUT_F :][::-1]            # [1024]

    # Pack per-core operands. w_eff[(i,k'),(n,p)] -> wt[i][p, (n,k')]
    wt_all = w_eff.reshape(N_CORES, KPC, NCHUNK, 128).transpose(0, 3, 2, 1)
    wt_all = np.ascontiguousarray(
        wt_all.reshape(N_CORES, 128, NCHUNK * KPC)
    ).astype(ml_dtypes.bfloat16)
    # x[b, (n,p)] -> xt[p, (n,b)], replicated
    xt = np.ascontiguousarray(
        x.reshape(B, NCHUNK, 128).transpose(2, 1, 0).reshape(128, NCHUNK * B)
    ).astype(ml_dtypes.bfloat16)
    b_all = np.ascontiguousarray(b_eff.reshape(N_CORES, KPC, 1))

    in_maps = [
        {"wt": wt_all[i], "xt": xt, "bi": b_all[i]} for i in range(N_CORES)
    ]
    LAST_RESULT = run_bass_kernel_spmd(_NC, in_maps, list(range(N_CORES)))

    # Gather: core i returns out[k', b] for k = i*KPC + k'.
    out_t = np.concatenate([r["out"] for r in LAST_RESULT.results], axis=0)
    return np.ascontiguousarray(out_t.T)
